# revision 21
# baseline (speedup 1.0000x reference)
"""CKConv (SIREN-generated causal conv1d) Trainium2 kernel.

Problem: x[B=4, Cin=32, L=2048]; a tiny SIREN MLP generates a conv kernel
[Cout=32, Cin=32, L]; output = causal conv + bias -> [4, 32, 2048].

Strategy:
  - Host: run the (negligible, O(H*L)) SIREN generator in numpy with
    REVERSED positions, producing the flipped kernel Wf[o,i,j'] directly
    (out[b,o,t] = sum_{i,j'<=t} Wf[o,i,j'] * x[b,i,t-j']), and pre-shuffle
    it into matmul tile layout.
  - Device (8 NeuronCores, SPMD): core k handles batch b=k//2 and the
    tap-parity half h=k%2 (alternating 16-tap blocks); the two cores of a
    pair produce partial sums the host adds (2M flops).  The causal conv
    is dense 128x128xN TensorE matmuls:
      K = 128 = (dj in 4) x (i in 32)   -- im2col: 4 time-shifted x copies
      M = 128 = (g in 4) x (o in 32)    -- 4 tap-groups per call
      N <= 512                          -- one PSUM bank per output tile
    Local call l covers taps j' = 32l + 16h + 4g + dj.  For output tile T
    (512 cols), calls l=0..16(T+1)-1 accumulate in PSUM; calls past the
    causal boundary are column-trimmed (their leading columns only touch
    zero padding).  Tap-group g lands shifted by 4g columns; VectorE folds
    the four 32-partition-aligned blocks into a [32, 2048] accumulator,
    ScalarE adds bias (h=0 core only), DMA out.
  - Weights stream just-in-time in 8 chunks; warmup matmuls during the
    initial DMA keep the PE HAM clock at 2.4 GHz.
"""

import numpy as np

import concourse.mybir as mybir
import concourse.tile as tile
from concourse import bacc
from concourse.bass_utils import run_bass_kernel_spmd

B, CIN, COUT, L, HID = 4, 32, 32, 2048, 32
OMEGA = 30.0
NCORES = 8
PAD = 512            # xim left zero padding (covers max lookback 16*31+496)
XIMW = PAD + L       # 2560
NT = 4               # output tiles of 512
NCH = 64             # weight tiles (local calls) per core; 16 taps each
NWCH = 8             # weight DMA chunks (8 calls each)
NWARM = 7            # PE warmup matmuls (bf16, ~430ns each cold)

KDTYPE = "fp16"      # "fp16" | "f32r"

TRACE = False
LAST_EXEC_NS = None
LAST_RESULTS = None

_NC = {}


def _build_nc(kdtype):
    nc = bacc.Bacc(None, target_bir_lowering=False)
    f32 = mybir.dt.float32
    dt = mybir.dt.float16 if kdtype == "fp16" else mybir.dt.float32r
    # x is host-padded: [4+PAD zero cols | x], so no on-device memset is
    # needed; the dj-th im2col block reads xin[:, 4-dj : 4-dj+XIMW].
    xin = nc.dram_tensor("xin", [CIN, 4 + XIMW], dt, kind="ExternalInput")
    wd = nc.dram_tensor("w", [128, NCH, 128], dt, kind="ExternalInput")
    bd = nc.dram_tensor("bias", [COUT, 1], f32, kind="ExternalInput")
    od = nc.dram_tensor("out", [COUT, L], f32, kind="ExternalOutput")

    with tile.TileContext(nc) as tc:
        with (
            tc.tile_pool(name="const", bufs=1) as cpool,
            tc.tile_pool(name="ps", bufs=2, space="PSUM") as pspool,
            tc.tile_pool(name="pswarm", bufs=1, space="PSUM") as pswarm,
            tc.tile_pool(name="stage", bufs=2) as spool,
        ):
            # PE warmup: bf16 matmuls on a zeroed dummy tile into a scratch
            # PSUM bank that is never read.  No input deps, so they run
            # while the DMAs stream, lifting the HAM clock gate to 2.4 GHz
            # before the real matmuls start.
            dummy = cpool.tile([128, 512], mybir.dt.bfloat16)
            nc.vector.memset(dummy[:], 0.0)
            wps = pswarm.tile([128, 512], f32)
            for _ in range(NWARM):
                nc.tensor.matmul(
                    wps[:], dummy[:, 0:128], dummy[:], start=True, stop=True
                )

            # im2col input: xim[dj*32 + i, PAD + t + dj] = x[i, t]
            xim = cpool.tile([128, XIMW], dt)
            for dj in range(4):
                nc.sync.dma_start(
                    out=xim[dj * 32 : (dj + 1) * 32, :],
                    in_=xin[:, 4 - dj : 4 - dj + XIMW],
                )

            # weight tiles, NWCH chunks streamed just-in-time
            # weight DMAs go on the second HWDGE ring (scalar) so their
            # issue overlaps the xim DMAs on the sync ring
            cs = NCH // NWCH
            wch = []
            for t in range(NWCH):
                wt = cpool.tile([128, cs, 128], dt, tag=f"w{t}")
                nc.scalar.dma_start(out=wt[:], in_=wd[:, cs * t : cs * (t + 1), :])
                wch.append(wt)

            bias_sb = cpool.tile([COUT, 1], f32)
            nc.sync.dma_start(out=bias_sb[:], in_=bd[:])

            acc = cpool.tile([COUT, L + 64], f32)
            nc.vector.memset(acc[:], 0.0)

            for T in range(NT):
                ps = pspool.tile([128, 512], f32)
                ncalls = 16 * (T + 1)
                for l in range(ncalls):
                    s = PAD + 512 * T - 32 * l
                    # columns below n0 only touch the zero padding -> trim
                    n0 = max(0, 32 * (l - 16 * T))
                    nc.tensor.matmul(
                        ps[:, n0:512],
                        wch[l // cs][:, l % cs, :],
                        xim[:, s + n0 : s + 512],
                        start=(l == 0),
                        stop=(l == ncalls - 1),
                    )
                # fold tap-groups: psum[g*32+o, n] -> out[o, 512T + n + 4g]
                # (T=3's spill adds land in acc's pad columns, never read)
                for g in range(4):
                    nc.vector.tensor_add(
                        out=acc[:, 512 * T + 4 * g : 512 * T + 512],
                        in0=acc[:, 512 * T + 4 * g : 512 * T + 512],
                        in1=ps[32 * g : 32 * g + 32, 0 : 512 - 4 * g],
                    )
                for g in range(1, 4):
                    nc.vector.tensor_add(
                        out=acc[:, 512 * (T + 1) : 512 * (T + 1) + 4 * g],
                        in0=acc[:, 512 * (T + 1) : 512 * (T + 1) + 4 * g],
                        in1=ps[32 * g : 32 * g + 32, 512 - 4 * g : 512],
                    )
                ft = spool.tile([COUT, 512], f32, tag="ft")
                nc.scalar.activation(
                    ft[:],
                    acc[:, 512 * T : 512 * T + 512],
                    mybir.ActivationFunctionType.Identity,
                    bias=bias_sb[:],
                )
                nc.sync.dma_start(out=od[:, 512 * T : 512 * T + 512], in_=ft[:])

    nc.compile()
    return nc


def _gen_flipped_kernel(w1, b1, w2, b2, w3, b3):
    """SIREN generator with reversed positions -> Wf[o, i, j'] = k[o, i, L-1-j']."""
    pos = np.linspace(-1.0, 1.0, L, dtype=np.float32)[::-1].astype(np.float64)
    w1 = w1.astype(np.float64)
    w2 = w2.astype(np.float64)
    w3 = w3.astype(np.float64)
    h = np.sin(OMEGA * (w1[:, 0][:, None] * pos[None, :] + b1.astype(np.float64)[:, None]))
    h = np.sin(OMEGA * (w2 @ h + b2.astype(np.float64)[:, None]))
    k = w3 @ h + b3.astype(np.float64)[:, None]
    return k.reshape(COUT, CIN, L).astype(np.float32)


def _shuffle_weights(wf, npdt):
    """wf[o,i,j'] -> per tap-parity-half h: wt[p=dj*32+i, l, m=g*32+o]
    = wf[o, i, 32l + 16h + 4g + dj].

    The device pairs call l's weights with x-window column
    512T + n - 32l - dj and the unpack maps psum col n to
    out t = 512T + n + 4g.  With tap J = 32l + 16h + 4g + dj the correct
    x index is t - J = 512T + n - 32l - dj - 16h: the h=1 core therefore
    receives its input shifted right by 16 columns (see kernel()), which
    makes the device program identical on all cores.
    """
    outs = []
    for h in range(2):
        v = wf.reshape(COUT, CIN, NCH, 2, 4, 4)[:, :, :, h]   # [o,i,l,g,dj]
        v = v.transpose(4, 1, 2, 3, 0)                        # [dj,i,l,g,o]
        outs.append(np.ascontiguousarray(v.reshape(128, NCH, 128).astype(npdt)))
    return outs


def kernel(x, w1, b1, w2, b2, w3, b3, bias):
    global LAST_EXEC_NS, LAST_RESULTS
    x = np.ascontiguousarray(np.asarray(x, dtype=np.float32))
    bias = np.asarray(bias, dtype=np.float32)
    npdt = np.float16 if KDTYPE == "fp16" else np.float32

    wf = _gen_flipped_kernel(
        np.asarray(w1), np.asarray(b1), np.asarray(w2), np.asarray(b2),
        np.asarray(w3), np.asarray(b3),
    )  # [COUT, CIN, L]
    wds = _shuffle_weights(wf, npdt)

    if KDTYPE not in _NC:
        _NC[KDTYPE] = _build_nc(KDTYPE)

    # host-side zero padding (4 + PAD cols); h=1 cores see x shifted right
    # by 16 (their taps are 16 later)
    xp0 = np.zeros((B, CIN, 4 + XIMW), dtype=npdt)
    xp0[:, :, 4 + PAD :] = x.astype(npdt)
    xp1 = np.zeros((B, CIN, 4 + XIMW), dtype=npdt)
    xp1[:, :, 4 + PAD + 16 :] = x[:, :, :-16].astype(npdt)

    bias0 = np.ascontiguousarray(bias.reshape(COUT, 1))
    bias1 = np.zeros((COUT, 1), dtype=np.float32)

    in_maps = []
    for k in range(NCORES):
        b, h = k // 2, k % 2
        in_maps.append(
            {
                "xin": xp0[b] if h == 0 else xp1[b],
                "w": wds[h],
                "bias": bias0 if h == 0 else bias1,
            }
        )

    res = run_bass_kernel_spmd(
        _NC[KDTYPE], in_maps, core_ids=list(range(NCORES)), trace=TRACE
    )
    LAST_RESULTS = res
    LAST_EXEC_NS = res.exec_time_ns

    out = np.empty((B, COUT, L), dtype=np.float32)
    for b in range(B):
        out[b] = res.results[2 * b]["out"] + res.results[2 * b + 1]["out"]
    return out


# revision 27
# speedup vs baseline: 1.1094x; 1.1094x over previous
"""CKConv (SIREN-generated causal conv1d) Trainium2 kernel.

Problem: x[B=4, Cin=32, L=2048]; a tiny SIREN MLP generates a conv kernel
[Cout=32, Cin=32, L]; output = causal conv + bias -> [4, 32, 2048].

Strategy:
  - Host: run the (negligible, O(H*L)) SIREN generator in numpy with
    REVERSED positions, producing the flipped kernel Wf[o,i,j'] directly
    (out[b,o,t] = sum_{i,j'<=t} Wf[o,i,j'] * x[b,i,t-j']), and pre-shuffle
    it into matmul tile layout.
  - Device (8 NeuronCores, SPMD): core k handles batch b=k//2 and the
    tap-parity half h=k%2 (alternating 16-tap blocks); the two cores of a
    pair produce partial sums the host adds (2M flops).  The causal conv
    is dense 128x128xN TensorE matmuls:
      K = 128 = (dj in 4) x (i in 32)   -- im2col: 4 time-shifted x copies
      M = 128 = (g in 4) x (o in 32)    -- 4 tap-groups per call
      N <= 512                          -- one PSUM bank per output tile
    Local call l covers taps j' = 32l + 16h + 4g + dj.  For output tile T
    (512 cols), calls l=0..16(T+1)-1 accumulate in PSUM; calls past the
    causal boundary are column-trimmed (their leading columns only touch
    zero padding).  Tap-group g lands shifted by 4g columns; VectorE folds
    the four 32-partition-aligned blocks into a [32, 2048] accumulator,
    ScalarE adds bias (h=0 core only), DMA out.
  - Weights stream just-in-time in 8 chunks; warmup matmuls during the
    initial DMA keep the PE HAM clock at 2.4 GHz.
"""

import numpy as np

import concourse.mybir as mybir
import concourse.tile as tile
from concourse import bacc
from concourse.bass_utils import run_bass_kernel_spmd

B, CIN, COUT, L, HID = 4, 32, 32, 2048, 32
OMEGA = 30.0
NCORES = 8
PAD = 512            # xim left zero padding (covers max lookback 16*31+496)
XIMW = PAD + L       # 2560
NT = 4               # output tiles of 512
NCH = 64             # weight tiles (local calls) per core; 16 taps each
NWCH = 4             # weight DMA chunks (16 calls each)
NWARM = 6            # PE warmup matmuls (bf16, ~430-630ns each cold)

KDTYPE = "fp16"      # "fp16" | "f32r"

TRACE = False
LAST_EXEC_NS = None
LAST_RESULTS = None

_NC = {}


def _build_nc(kdtype):
    nc = bacc.Bacc(None, target_bir_lowering=False)
    f32 = mybir.dt.float32
    dt = mybir.dt.float16 if kdtype == "fp16" else mybir.dt.float32r
    # host-prebuilt im2col image: xin[dj*32+i, PAD + t + dj] = x[i, t],
    # zeros elsewhere -- one 128-partition DMA, no on-device memset
    xin = nc.dram_tensor("xin", [128, XIMW], dt, kind="ExternalInput")
    wd = nc.dram_tensor("w", [128, NCH, 128], dt, kind="ExternalInput")
    bd = nc.dram_tensor("bias", [COUT, 1], f32, kind="ExternalInput")
    od = nc.dram_tensor("out", [COUT, L], f32, kind="ExternalOutput")

    with tile.TileContext(nc) as tc:
        with (
            tc.tile_pool(name="const", bufs=1) as cpool,
            tc.tile_pool(name="ps", bufs=2, space="PSUM") as pspool,
            tc.tile_pool(name="pswarm", bufs=1, space="PSUM") as pswarm,
            tc.tile_pool(name="stage", bufs=2) as spool,
        ):
            # PE warmup: bf16 matmuls on a zeroed dummy tile into a scratch
            # PSUM bank that is never read.  No input deps, so they run
            # while the DMAs stream, lifting the HAM clock gate to 2.4 GHz
            # before the real matmuls start.
            dummy = cpool.tile([128, 512], mybir.dt.bfloat16)
            nc.vector.memset(dummy[:], 0.0)
            wps = pswarm.tile([128, 512], f32)
            for _ in range(NWARM):
                nc.tensor.matmul(
                    wps[:], dummy[:, 0:128], dummy[:], start=True, stop=True
                )

            # im2col input: xim[dj*32 + i, PAD + t + dj] = x[i, t]
            xim = cpool.tile([128, XIMW], dt)
            nc.sync.dma_start(out=xim[:], in_=xin[:])

            # weight tiles, NWCH chunks streamed just-in-time
            # weight DMAs go on the second HWDGE ring (scalar) so their
            # issue overlaps the xim DMAs on the sync ring
            cs = NCH // NWCH
            wch = []
            for t in range(NWCH):
                wt = cpool.tile([128, cs, 128], dt, tag=f"w{t}")
                nc.scalar.dma_start(out=wt[:], in_=wd[:, cs * t : cs * (t + 1), :])
                wch.append(wt)

            bias_sb = cpool.tile([COUT, 1], f32)
            nc.sync.dma_start(out=bias_sb[:], in_=bd[:])

            acc = cpool.tile([COUT, L + 64], f32)
            nc.vector.memset(acc[:], 0.0)

            for T in range(NT):
                ps = pspool.tile([128, 512], f32)
                ncalls = 16 * (T + 1)
                for l in range(ncalls):
                    s = PAD + 512 * T - 32 * l
                    # columns below n0 only touch the zero padding -> trim
                    n0 = max(0, 32 * (l - 16 * T))
                    nc.tensor.matmul(
                        ps[:, n0:512],
                        wch[l // cs][:, l % cs, :],
                        xim[:, s + n0 : s + 512],
                        start=(l == 0),
                        stop=(l == ncalls - 1),
                    )
                # fold tap-groups: psum[g*32+o, n] -> out[o, 512T + n + 4g]
                # (T=3's spill adds land in acc's pad columns, never read)
                for g in range(4):
                    nc.vector.tensor_add(
                        out=acc[:, 512 * T + 4 * g : 512 * T + 512],
                        in0=acc[:, 512 * T + 4 * g : 512 * T + 512],
                        in1=ps[32 * g : 32 * g + 32, 0 : 512 - 4 * g],
                    )
                for g in range(1, 4):
                    nc.vector.tensor_add(
                        out=acc[:, 512 * (T + 1) : 512 * (T + 1) + 4 * g],
                        in0=acc[:, 512 * (T + 1) : 512 * (T + 1) + 4 * g],
                        in1=ps[32 * g : 32 * g + 32, 512 - 4 * g : 512],
                    )
                ft = spool.tile([COUT, 512], f32, tag="ft")
                nc.scalar.activation(
                    ft[:],
                    acc[:, 512 * T : 512 * T + 512],
                    mybir.ActivationFunctionType.Identity,
                    bias=bias_sb[:],
                )
                nc.sync.dma_start(out=od[:, 512 * T : 512 * T + 512], in_=ft[:])

    nc.compile()
    return nc


def _gen_flipped_kernel(w1, b1, w2, b2, w3, b3):
    """SIREN generator with reversed positions -> Wf[o, i, j'] = k[o, i, L-1-j']."""
    pos = np.linspace(-1.0, 1.0, L, dtype=np.float32)[::-1].astype(np.float64)
    w1 = w1.astype(np.float64)
    w2 = w2.astype(np.float64)
    w3 = w3.astype(np.float64)
    h = np.sin(OMEGA * (w1[:, 0][:, None] * pos[None, :] + b1.astype(np.float64)[:, None]))
    h = np.sin(OMEGA * (w2 @ h + b2.astype(np.float64)[:, None]))
    k = w3 @ h + b3.astype(np.float64)[:, None]
    return k.reshape(COUT, CIN, L).astype(np.float32)


def _shuffle_weights(wf, npdt):
    """wf[o,i,j'] -> per tap-parity-half h: wt[p=dj*32+i, l, m=g*32+o]
    = wf[o, i, 32l + 16h + 4g + dj].

    The device pairs call l's weights with x-window column
    512T + n - 32l - dj and the unpack maps psum col n to
    out t = 512T + n + 4g.  With tap J = 32l + 16h + 4g + dj the correct
    x index is t - J = 512T + n - 32l - dj - 16h: the h=1 core therefore
    receives its input shifted right by 16 columns (see kernel()), which
    makes the device program identical on all cores.
    """
    outs = []
    for h in range(2):
        v = wf.reshape(COUT, CIN, NCH, 2, 4, 4)[:, :, :, h]   # [o,i,l,g,dj]
        v = v.transpose(4, 1, 2, 3, 0)                        # [dj,i,l,g,o]
        outs.append(np.ascontiguousarray(v.reshape(128, NCH, 128).astype(npdt)))
    return outs


def kernel(x, w1, b1, w2, b2, w3, b3, bias):
    global LAST_EXEC_NS, LAST_RESULTS
    x = np.ascontiguousarray(np.asarray(x, dtype=np.float32))
    bias = np.asarray(bias, dtype=np.float32)
    npdt = np.float16 if KDTYPE == "fp16" else np.float32

    wf = _gen_flipped_kernel(
        np.asarray(w1), np.asarray(b1), np.asarray(w2), np.asarray(b2),
        np.asarray(w3), np.asarray(b3),
    )  # [COUT, CIN, L]
    wds = _shuffle_weights(wf, npdt)

    if KDTYPE not in _NC:
        _NC[KDTYPE] = _build_nc(KDTYPE)

    # host-built im2col images: xim[dj*32+i, PAD+dj+t] = xc[i, t] where
    # xc = x for h=0 and x shifted right by 16 for h=1 (its taps are 16
    # later); columns beyond XIMW are never read and simply dropped
    xh = x.astype(npdt)
    xims = np.zeros((B, 2, 128, XIMW), dtype=npdt)
    for dj in range(4):
        blk = slice(32 * dj, 32 * dj + 32)
        xims[:, 0, blk, PAD + dj : XIMW] = xh[:, :, : L - dj]
        xims[:, 1, blk, PAD + dj + 16 : XIMW] = xh[:, :, : L - dj - 16]

    bias0 = np.ascontiguousarray(bias.reshape(COUT, 1))
    bias1 = np.zeros((COUT, 1), dtype=np.float32)

    in_maps = []
    for k in range(NCORES):
        b, h = k // 2, k % 2
        in_maps.append(
            {
                "xin": xims[b, h],
                "w": wds[h],
                "bias": bias0 if h == 0 else bias1,
            }
        )

    res = run_bass_kernel_spmd(
        _NC[KDTYPE], in_maps, core_ids=list(range(NCORES)), trace=TRACE
    )
    LAST_RESULTS = res
    LAST_EXEC_NS = res.exec_time_ns

    out = np.empty((B, COUT, L), dtype=np.float32)
    for b in range(B):
        out[b] = res.results[2 * b]["out"] + res.results[2 * b + 1]["out"]
    return out


# revision 30
# speedup vs baseline: 1.1512x; 1.0377x over previous
"""CKConv (SIREN-generated causal conv1d) Trainium2 kernel.

Problem: x[B=4, Cin=32, L=2048]; a tiny SIREN MLP generates a conv kernel
[Cout=32, Cin=32, L]; output = causal conv + bias -> [4, 32, 2048].

Strategy:
  - Host: run the (negligible, O(H*L)) SIREN generator in numpy with
    REVERSED positions, producing the flipped kernel Wf[o,i,j'] directly
    (out[b,o,t] = sum_{i,j'<=t} Wf[o,i,j'] * x[b,i,t-j']), and pre-shuffle
    it into matmul tile layout.
  - Device (8 NeuronCores, SPMD): core k handles batch b=k//2 and the
    tap-parity half h=k%2 (alternating 16-tap blocks); the two cores of a
    pair produce partial sums the host adds (2M flops).  The causal conv
    is dense 128x128xN TensorE matmuls:
      K = 128 = (dj in 4) x (i in 32)   -- im2col: 4 time-shifted x copies
      M = 128 = (g in 4) x (o in 32)    -- 4 tap-groups per call
      N <= 512                          -- one PSUM bank per output tile
    Local call l covers taps j' = 32l + 16h + 4g + dj.  For output tile T
    (512 cols), calls l=0..16(T+1)-1 accumulate in PSUM; calls past the
    causal boundary are column-trimmed (their leading columns only touch
    zero padding).  Tap-group g lands shifted by 4g columns; VectorE folds
    the four 32-partition-aligned blocks into a [32, 2048] accumulator,
    ScalarE adds bias (h=0 core only), DMA out.
  - Weights stream just-in-time in 8 chunks; warmup matmuls during the
    initial DMA keep the PE HAM clock at 2.4 GHz.
"""

import numpy as np

import concourse.mybir as mybir
import concourse.tile as tile
from concourse import bacc
from concourse.bass_utils import run_bass_kernel_spmd

B, CIN, COUT, L, HID = 4, 32, 32, 2048, 32
OMEGA = 30.0
NCORES = 8
PAD = 512            # xim left zero padding (covers max lookback 16*31+496)
XIMW = PAD + L       # 2560
NT = 4               # output tiles of 512
NCH = 64             # weight tiles (local calls) per core; 16 taps each
NWCH = 4             # weight DMA chunks (16 calls each)
NWARM = 8            # PE warmup matmuls (bf16, ~630ns each cold)

KDTYPE = "fp16"      # "fp16" | "f32r"

TRACE = False
LAST_EXEC_NS = None
LAST_RESULTS = None

_NC = {}


def _build_nc(kdtype):
    nc = bacc.Bacc(None, target_bir_lowering=False)
    f32 = mybir.dt.float32
    dt = mybir.dt.float16 if kdtype == "fp16" else mybir.dt.float32r
    # host-prebuilt im2col image: xin[dj*32+i, PAD + t + dj] = x[i, t],
    # zeros elsewhere -- one 128-partition DMA, no on-device memset
    xin = nc.dram_tensor("xin", [128, XIMW], dt, kind="ExternalInput")
    wd = nc.dram_tensor("w", [128, NCH, 128], dt, kind="ExternalInput")
    bd = nc.dram_tensor("bias", [COUT, 1], f32, kind="ExternalInput")
    od = nc.dram_tensor("out", [COUT, L], f32, kind="ExternalOutput")

    with tile.TileContext(nc) as tc:
        with (
            tc.tile_pool(name="const", bufs=1) as cpool,
            tc.tile_pool(name="ps", bufs=2, space="PSUM") as pspool,
            tc.tile_pool(name="pswarm", bufs=1, space="PSUM") as pswarm,
            tc.tile_pool(name="stage", bufs=2) as spool,
        ):
            # PE warmup: bf16 matmuls on a zeroed dummy tile into a scratch
            # PSUM bank that is never read.  No input deps, so they run
            # while the DMAs stream, lifting the HAM clock gate to 2.4 GHz
            # before the real matmuls start.
            dummy = cpool.tile([128, 512], mybir.dt.bfloat16)
            nc.vector.memset(dummy[:], 0.0)
            wps = pswarm.tile([128, 512], f32)
            for _ in range(NWARM):
                nc.tensor.matmul(
                    wps[:], dummy[:, 0:128], dummy[:], start=True, stop=True
                )

            # im2col input: xim[dj*32 + i, PAD + t + dj] = x[i, t]
            xim = cpool.tile([128, XIMW], dt)
            nc.sync.dma_start(out=xim[:], in_=xin[:])

            # weight tiles, NWCH chunks streamed just-in-time
            # weight DMAs go on the second HWDGE ring (scalar) so their
            # issue overlaps the xim DMAs on the sync ring
            cs = NCH // NWCH
            wch = []
            for t in range(NWCH):
                wt = cpool.tile([128, cs, 128], dt, tag=f"w{t}")
                nc.scalar.dma_start(out=wt[:], in_=wd[:, cs * t : cs * (t + 1), :])
                wch.append(wt)

            bias_sb = cpool.tile([COUT, 1], f32)
            nc.sync.dma_start(out=bias_sb[:], in_=bd[:])

            # acc starts as broadcast(bias): the per-tile output is then
            # just a DMA of the finished acc slice (no ScalarE pass)
            acc = cpool.tile([COUT, L + 64], f32)
            nc.vector.memset(acc[:], 0.0)
            nc.scalar.activation(
                acc[:],
                acc[:],
                mybir.ActivationFunctionType.Identity,
                bias=bias_sb[:],
            )

            for T in range(NT):
                ps = pspool.tile([128, 512], f32)
                ncalls = 16 * (T + 1)
                for l in range(ncalls):
                    s = PAD + 512 * T - 32 * l
                    # columns below n0 only touch the zero padding -> trim
                    n0 = max(0, 32 * (l - 16 * T))
                    nc.tensor.matmul(
                        ps[:, n0:512],
                        wch[l // cs][:, l % cs, :],
                        xim[:, s + n0 : s + 512],
                        start=(l == 0),
                        stop=(l == ncalls - 1),
                    )
                # fold tap-groups: psum[g*32+o, n] -> out[o, 512T + n + 4g]
                # (T=3's spill adds land in acc's pad columns, never read)
                for g in range(4):
                    nc.vector.tensor_add(
                        out=acc[:, 512 * T + 4 * g : 512 * T + 512],
                        in0=acc[:, 512 * T + 4 * g : 512 * T + 512],
                        in1=ps[32 * g : 32 * g + 32, 0 : 512 - 4 * g],
                    )
                for g in range(1, 4):
                    nc.vector.tensor_add(
                        out=acc[:, 512 * (T + 1) : 512 * (T + 1) + 4 * g],
                        in0=acc[:, 512 * (T + 1) : 512 * (T + 1) + 4 * g],
                        in1=ps[32 * g : 32 * g + 32, 512 - 4 * g : 512],
                    )
                nc.sync.dma_start(
                    out=od[:, 512 * T : 512 * T + 512],
                    in_=acc[:, 512 * T : 512 * T + 512],
                )

    nc.compile()
    return nc


def _gen_flipped_kernel(w1, b1, w2, b2, w3, b3):
    """SIREN generator with reversed positions -> Wf[o, i, j'] = k[o, i, L-1-j']."""
    pos = np.linspace(-1.0, 1.0, L, dtype=np.float32)[::-1].astype(np.float64)
    w1 = w1.astype(np.float64)
    w2 = w2.astype(np.float64)
    w3 = w3.astype(np.float64)
    h = np.sin(OMEGA * (w1[:, 0][:, None] * pos[None, :] + b1.astype(np.float64)[:, None]))
    h = np.sin(OMEGA * (w2 @ h + b2.astype(np.float64)[:, None]))
    k = w3 @ h + b3.astype(np.float64)[:, None]
    return k.reshape(COUT, CIN, L).astype(np.float32)


def _shuffle_weights(wf, npdt):
    """wf[o,i,j'] -> per tap-parity-half h: wt[p=dj*32+i, l, m=g*32+o]
    = wf[o, i, 32l + 16h + 4g + dj].

    The device pairs call l's weights with x-window column
    512T + n - 32l - dj and the unpack maps psum col n to
    out t = 512T + n + 4g.  With tap J = 32l + 16h + 4g + dj the correct
    x index is t - J = 512T + n - 32l - dj - 16h: the h=1 core therefore
    receives its input shifted right by 16 columns (see kernel()), which
    makes the device program identical on all cores.
    """
    outs = []
    for h in range(2):
        v = wf.reshape(COUT, CIN, NCH, 2, 4, 4)[:, :, :, h]   # [o,i,l,g,dj]
        v = v.transpose(4, 1, 2, 3, 0)                        # [dj,i,l,g,o]
        outs.append(np.ascontiguousarray(v.reshape(128, NCH, 128).astype(npdt)))
    return outs


def kernel(x, w1, b1, w2, b2, w3, b3, bias):
    global LAST_EXEC_NS, LAST_RESULTS
    x = np.ascontiguousarray(np.asarray(x, dtype=np.float32))
    bias = np.asarray(bias, dtype=np.float32)
    npdt = np.float16 if KDTYPE == "fp16" else np.float32

    wf = _gen_flipped_kernel(
        np.asarray(w1), np.asarray(b1), np.asarray(w2), np.asarray(b2),
        np.asarray(w3), np.asarray(b3),
    )  # [COUT, CIN, L]
    wds = _shuffle_weights(wf, npdt)

    if KDTYPE not in _NC:
        _NC[KDTYPE] = _build_nc(KDTYPE)

    # host-built im2col images: xim[dj*32+i, PAD+dj+t] = xc[i, t] where
    # xc = x for h=0 and x shifted right by 16 for h=1 (its taps are 16
    # later); columns beyond XIMW are never read and simply dropped
    xh = x.astype(npdt)
    xims = np.zeros((B, 2, 128, XIMW), dtype=npdt)
    for dj in range(4):
        blk = slice(32 * dj, 32 * dj + 32)
        xims[:, 0, blk, PAD + dj : XIMW] = xh[:, :, : L - dj]
        xims[:, 1, blk, PAD + dj + 16 : XIMW] = xh[:, :, : L - dj - 16]

    bias0 = np.ascontiguousarray(bias.reshape(COUT, 1))
    bias1 = np.zeros((COUT, 1), dtype=np.float32)

    in_maps = []
    for k in range(NCORES):
        b, h = k // 2, k % 2
        in_maps.append(
            {
                "xin": xims[b, h],
                "w": wds[h],
                "bias": bias0 if h == 0 else bias1,
            }
        )

    res = run_bass_kernel_spmd(
        _NC[KDTYPE], in_maps, core_ids=list(range(NCORES)), trace=TRACE
    )
    LAST_RESULTS = res
    LAST_EXEC_NS = res.exec_time_ns

    out = np.empty((B, COUT, L), dtype=np.float32)
    for b in range(B):
        out[b] = res.results[2 * b]["out"] + res.results[2 * b + 1]["out"]
    return out


# revision 38
# speedup vs baseline: 1.1850x; 1.0294x over previous
"""CKConv (SIREN-generated causal conv1d) Trainium2 kernel.

Problem: x[B=4, Cin=32, L=2048]; a tiny SIREN MLP generates a conv kernel
[Cout=32, Cin=32, L]; output = causal conv + bias -> [4, 32, 2048].

Strategy:
  - Host: run the (negligible, O(H*L)) SIREN generator in numpy with
    REVERSED positions, producing the flipped kernel Wf[o,i,j'] directly
    (out[b,o,t] = sum_{i,j'<=t} Wf[o,i,j'] * x[b,i,t-j']), and pre-shuffle
    it into matmul tile layout.
  - Device (8 NeuronCores, SPMD): core k handles batch b=k//2 and the
    tap-parity half h=k%2 (alternating 16-tap blocks); the two cores of a
    pair produce partial sums the host adds (2M flops).  The causal conv
    is dense 128x128xN TensorE matmuls:
      K = 128 = (dj in 4) x (i in 32)   -- im2col: 4 time-shifted x copies
      M = 128 = (g in 4) x (o in 32)    -- 4 tap-groups per call
      N <= 512                          -- one PSUM bank per output tile
    Local call l covers taps j' = 32l + 16h + 4g + dj.  For output tile T
    (512 cols), calls l=0..16(T+1)-1 accumulate in PSUM; calls past the
    causal boundary are column-trimmed (their leading columns only touch
    zero padding).  Tap-group g lands shifted by 4g columns; VectorE folds
    the four 32-partition-aligned blocks into a [32, 2048] accumulator
    pre-initialized with the bias (h=0 core only), DMA out.
  - Weights stream just-in-time in growing chunks chained into a DMA
    ladder (so early HBM bandwidth goes to first-needed data); warmup
    matmuls during the initial DMA keep the PE HAM clock at 2.4 GHz.
  - KDTYPE "fp16" is the production path (~2e-3 scale-relative error,
    ~49us).  "f32r" (~9e-4, ~+6us) predates the DMA ladder and would need
    the ladder probes removed to compile.
"""

import numpy as np

import concourse.mybir as mybir
import concourse.tile as tile
from concourse import bacc
from concourse.bass_utils import run_bass_kernel_spmd

B, CIN, COUT, L, HID = 4, 32, 32, 2048, 32
OMEGA = 30.0
NCORES = 8
# After causal trimming every matmul window starts at or after the data
# edge, so no left zero-padding is needed in the im2col buffer at all:
# xim[dj*32+i, t'] = x[i, t' - dj] (zeros for t' < dj), width exactly L.
XIMW = L             # 2048
NT = 4               # output tiles of 512
NCH = 64             # weight tiles (local calls) per core; 16 taps each
WCHUNKS = [8, 8, 16, 16, 16]   # weight DMA chunk sizes (calls)
NWARM = 5            # PE warmup matmuls (bf16 N=512, ~430-630ns each cold)

KDTYPE = "fp16"      # "fp16" | "f32r"

TRACE = False
LAST_EXEC_NS = None
LAST_RESULTS = None

_NC = {}


def _build_nc(kdtype):
    nc = bacc.Bacc(None, target_bir_lowering=False)
    f32 = mybir.dt.float32
    dt = mybir.dt.float16 if kdtype == "fp16" else mybir.dt.float32r
    # host-prebuilt im2col image: xin[dj*32+i, dj + t] = x[i, t],
    # zeros elsewhere -- one 128-partition DMA, no on-device memset
    xin = nc.dram_tensor("xin", [128, XIMW], dt, kind="ExternalInput")
    wd = nc.dram_tensor("w", [128, NCH, 128], dt, kind="ExternalInput")
    bd = nc.dram_tensor("bias", [COUT, 1], f32, kind="ExternalInput")
    od = nc.dram_tensor("out", [COUT, L], f32, kind="ExternalOutput")

    with tile.TileContext(nc) as tc:
        with (
            tc.tile_pool(name="const", bufs=1) as cpool,
            tc.tile_pool(name="ps", bufs=3, space="PSUM") as pspool,
            tc.tile_pool(name="pswarm", bufs=1, space="PSUM") as pswarm,
        ):
            # PE warmup: bf16 matmuls on a zeroed dummy tile into a scratch
            # PSUM bank that is never read.  No input deps, so they run
            # while the DMAs stream, lifting the HAM clock gate to 2.4 GHz
            # before the real matmuls start.
            dummy = cpool.tile([128, 512], mybir.dt.bfloat16)
            nc.vector.memset(dummy[:], 0.0)
            wps = pswarm.tile([128, 512], f32)
            for _ in range(NWARM):
                nc.tensor.matmul(
                    wps[:], dummy[:, 0:128], dummy[:], start=True, stop=True
                )

            # im2col input: xim[dj*32 + i, dj + t] = x[i, t]; split by
            # column range so tile 0's matmuls are gated on 0.125MB only
            xim = cpool.tile([128, XIMW], dt)
            nc.sync.dma_start(out=xim[:, 0:512], in_=xin[:, 0:512])
            nc.sync.dma_start(out=xim[:, 512:1024], in_=xin[:, 512:1024])
            nc.sync.dma_start(out=xim[:, 1024:2048], in_=xin[:, 1024:2048])

            # weight tiles streamed just-in-time; DMAs go on the second
            # HWDGE ring (scalar) so their
            # issue overlaps the xim DMA on the sync ring; chunk sizes grow
            # so the first matmuls are gated on as little data as possible
            wch = {}   # call l -> (tile, col index)
            c0 = 0
            prev = None
            for t, cs in enumerate(WCHUNKS):
                wt = cpool.tile([128, cs, 128], dt, tag=f"w{t}")
                if prev is not None:
                    # ladder: delay this chunk until the previous one is
                    # done (SDMA round-robin would otherwise steal early
                    # bandwidth from first-needed data); the probe write
                    # is overwritten by the DMA and only forces the dep
                    nc.vector.memset(wt[0:32, 0:1, 0:2].bitcast(mybir.dt.float32), 0.0)
                    nc.vector.tensor_copy(
                        wt[0:32, 0:1, 0:2].bitcast(mybir.dt.float32),
                        prev[0:32, 0:1, 0:2].bitcast(mybir.dt.float32),
                    )
                nc.scalar.dma_start(out=wt[:], in_=wd[:, c0 : c0 + cs, :])
                for j in range(cs):
                    wch[c0 + j] = (wt, j)
                c0 += cs
                prev = wt

            bias_sb = cpool.tile([COUT, 1], f32)
            nc.scalar.dma_start(out=bias_sb[:], in_=bd[:])

            # acc starts as broadcast(bias): the per-tile output is then
            # just a DMA of the finished acc slice (no ScalarE pass)
            acc = cpool.tile([COUT, L + 64], f32)
            nc.vector.memset(acc[:], 0.0)
            nc.scalar.activation(
                acc[:],
                acc[:],
                mybir.ActivationFunctionType.Identity,
                bias=bias_sb[:],
            )

            for T in range(NT):
                ps = pspool.tile([128, 512], f32)
                ncalls = 16 * (T + 1)
                for l in range(ncalls):
                    s = 512 * T - 32 * l
                    # columns below n0 only touch the zero padding -> trim
                    n0 = max(0, 32 * (l - 16 * T))
                    wt, j = wch[l]
                    nc.tensor.matmul(
                        ps[:, n0:512],
                        wt[:, j, :],
                        xim[:, s + n0 : s + 512],
                        start=(l == 0),
                        stop=(l == ncalls - 1),
                    )
                # fold tap-groups: psum[g*32+o, n] -> out[o, 512T + n + 4g];
                # each add spills up to 12 columns into the next tile's
                # region (or acc's pad for T=3), which that tile's DMA
                # picks up later -- exactly-once coverage per (col, g)
                for g in range(4):
                    nc.vector.tensor_add(
                        out=acc[:, 512 * T + 4 * g : 512 * T + 4 * g + 512],
                        in0=acc[:, 512 * T + 4 * g : 512 * T + 4 * g + 512],
                        in1=ps[32 * g : 32 * g + 32, :],
                    )
                nc.sync.dma_start(
                    out=od[:, 512 * T : 512 * T + 512],
                    in_=acc[:, 512 * T : 512 * T + 512],
                )

    nc.compile()
    return nc


def _gen_flipped_kernel(w1, b1, w2, b2, w3, b3):
    """SIREN generator with reversed positions -> Wf[o, i, j'] = k[o, i, L-1-j']."""
    pos = np.linspace(-1.0, 1.0, L, dtype=np.float32)[::-1].astype(np.float64)
    w1 = w1.astype(np.float64)
    w2 = w2.astype(np.float64)
    w3 = w3.astype(np.float64)
    h = np.sin(OMEGA * (w1[:, 0][:, None] * pos[None, :] + b1.astype(np.float64)[:, None]))
    h = np.sin(OMEGA * (w2 @ h + b2.astype(np.float64)[:, None]))
    k = w3 @ h + b3.astype(np.float64)[:, None]
    return k.reshape(COUT, CIN, L).astype(np.float32)


def _shuffle_weights(wf, npdt):
    """wf[o,i,j'] -> per tap-parity-half h: wt[p=dj*32+i, l, m=g*32+o]
    = wf[o, i, 32l + 16h + 4g + dj].

    The device pairs call l's weights with x-window column
    512T + n - 32l - dj and the unpack maps psum col n to
    out t = 512T + n + 4g.  With tap J = 32l + 16h + 4g + dj the correct
    x index is t - J = 512T + n - 32l - dj - 16h: the h=1 core therefore
    receives its input shifted right by 16 columns (see kernel()), which
    makes the device program identical on all cores.
    """
    outs = []
    for h in range(2):
        v = wf.reshape(COUT, CIN, NCH, 2, 4, 4)[:, :, :, h]   # [o,i,l,g,dj]
        v = v.transpose(4, 1, 2, 3, 0)                        # [dj,i,l,g,o]
        outs.append(np.ascontiguousarray(v.reshape(128, NCH, 128).astype(npdt)))
    return outs


def kernel(x, w1, b1, w2, b2, w3, b3, bias):
    global LAST_EXEC_NS, LAST_RESULTS
    x = np.ascontiguousarray(np.asarray(x, dtype=np.float32))
    bias = np.asarray(bias, dtype=np.float32)
    npdt = np.float16 if KDTYPE == "fp16" else np.float32

    wf = _gen_flipped_kernel(
        np.asarray(w1), np.asarray(b1), np.asarray(w2), np.asarray(b2),
        np.asarray(w3), np.asarray(b3),
    )  # [COUT, CIN, L]
    wds = _shuffle_weights(wf, npdt)

    if KDTYPE not in _NC:
        _NC[KDTYPE] = _build_nc(KDTYPE)

    # host-built im2col images: xim[dj*32+i, dj+t] = xc[i, t] where
    # xc = x for h=0 and x shifted right by 16 for h=1 (its taps are 16
    # later); columns beyond XIMW are never read and simply dropped
    xh = x.astype(npdt)
    xims = np.zeros((B, 2, 128, XIMW), dtype=npdt)
    for dj in range(4):
        blk = slice(32 * dj, 32 * dj + 32)
        xims[:, 0, blk, dj:XIMW] = xh[:, :, : L - dj]
        xims[:, 1, blk, dj + 16 : XIMW] = xh[:, :, : L - dj - 16]

    bias0 = np.ascontiguousarray(bias.reshape(COUT, 1))
    bias1 = np.zeros((COUT, 1), dtype=np.float32)

    in_maps = []
    for k in range(NCORES):
        b, h = k // 2, k % 2
        in_maps.append(
            {
                "xin": xims[b, h],
                "w": wds[h],
                "bias": bias0 if h == 0 else bias1,
            }
        )

    res = run_bass_kernel_spmd(
        _NC[KDTYPE], in_maps, core_ids=list(range(NCORES)), trace=TRACE
    )
    LAST_RESULTS = res
    LAST_EXEC_NS = res.exec_time_ns

    out = np.empty((B, COUT, L), dtype=np.float32)
    for b in range(B):
        out[b] = res.results[2 * b]["out"] + res.results[2 * b + 1]["out"]
    return out



# revision 39
# speedup vs baseline: 1.2063x; 1.0179x over previous
"""CKConv (SIREN-generated causal conv1d) Trainium2 kernel.

Problem: x[B=4, Cin=32, L=2048]; a tiny SIREN MLP generates a conv kernel
[Cout=32, Cin=32, L]; output = causal conv + bias -> [4, 32, 2048].

Strategy:
  - Host: run the (negligible, O(H*L)) SIREN generator in numpy with
    REVERSED positions, producing the flipped kernel Wf[o,i,j'] directly
    (out[b,o,t] = sum_{i,j'<=t} Wf[o,i,j'] * x[b,i,t-j']), and pre-shuffle
    it into matmul tile layout.
  - Device (8 NeuronCores, SPMD): core k handles batch b=k//2 and the
    tap-parity half h=k%2 (alternating 16-tap blocks); the two cores of a
    pair produce partial sums the host adds (2M flops).  The causal conv
    is dense 128x128xN TensorE matmuls:
      K = 128 = (dj in 4) x (i in 32)   -- im2col: 4 time-shifted x copies
      M = 128 = (g in 4) x (o in 32)    -- 4 tap-groups per call
      N <= 512                          -- one PSUM bank per output tile
    Local call l covers taps j' = 32l + 16h + 4g + dj.  For output tile T
    (512 cols), calls l=0..16(T+1)-1 accumulate in PSUM; calls past the
    causal boundary are column-trimmed (their leading columns only touch
    zero padding).  Tap-group g lands shifted by 4g columns; VectorE folds
    the four 32-partition-aligned blocks into a [32, 2048] accumulator
    pre-initialized with the bias (h=0 core only), DMA out.
  - Weights stream just-in-time in growing chunks chained into a DMA
    ladder (so early HBM bandwidth goes to first-needed data); warmup
    matmuls during the initial DMA keep the PE HAM clock at 2.4 GHz.
  - KDTYPE "fp16" is the production path (~2e-3 scale-relative error,
    ~49us).  "f32r" (~9e-4, ~+6us) predates the DMA ladder and would need
    the ladder probes removed to compile.
"""

import numpy as np

import concourse.mybir as mybir
import concourse.tile as tile
from concourse import bacc
from concourse.bass_utils import run_bass_kernel_spmd

B, CIN, COUT, L, HID = 4, 32, 32, 2048, 32
OMEGA = 30.0
NCORES = 8
# After causal trimming every matmul window starts at or after the data
# edge, so no left zero-padding is needed in the im2col buffer at all:
# xim[dj*32+i, t'] = x[i, t' - dj] (zeros for t' < dj), width exactly L.
XIMW = L             # 2048
NT = 4               # output tiles of 512
NCH = 64             # weight tiles (local calls) per core; 16 taps each
WCHUNKS = [4, 12, 16, 16, 16]  # weight DMA chunk sizes (calls)
NWARM = 5            # PE warmup matmuls (bf16 N=512, ~430-630ns each cold)

KDTYPE = "fp16"      # "fp16" | "f32r"

TRACE = False
LAST_EXEC_NS = None
LAST_RESULTS = None

_NC = {}


def _build_nc(kdtype):
    nc = bacc.Bacc(None, target_bir_lowering=False)
    f32 = mybir.dt.float32
    dt = mybir.dt.float16 if kdtype == "fp16" else mybir.dt.float32r
    # host-prebuilt im2col image: xin[dj*32+i, dj + t] = x[i, t],
    # zeros elsewhere -- one 128-partition DMA, no on-device memset
    xin = nc.dram_tensor("xin", [128, XIMW], dt, kind="ExternalInput")
    wd = nc.dram_tensor("w", [128, NCH, 128], dt, kind="ExternalInput")
    bd = nc.dram_tensor("bias", [COUT, 1], f32, kind="ExternalInput")
    od = nc.dram_tensor("out", [COUT, L], f32, kind="ExternalOutput")

    with tile.TileContext(nc) as tc:
        with (
            tc.tile_pool(name="const", bufs=1) as cpool,
            tc.tile_pool(name="ps", bufs=3, space="PSUM") as pspool,
            tc.tile_pool(name="pswarm", bufs=1, space="PSUM") as pswarm,
        ):
            # PE warmup: bf16 matmuls on a zeroed dummy tile into a scratch
            # PSUM bank that is never read.  No input deps, so they run
            # while the DMAs stream, lifting the HAM clock gate to 2.4 GHz
            # before the real matmuls start.
            dummy = cpool.tile([128, 512], mybir.dt.bfloat16)
            nc.vector.memset(dummy[:], 0.0)
            wps = pswarm.tile([128, 512], f32)
            for _ in range(NWARM):
                nc.tensor.matmul(
                    wps[:], dummy[:, 0:128], dummy[:], start=True, stop=True
                )

            # im2col input: xim[dj*32 + i, dj + t] = x[i, t]; split by
            # column range so tile 0's matmuls are gated on 0.125MB only
            xim = cpool.tile([128, XIMW], dt)
            nc.sync.dma_start(out=xim[:, 0:512], in_=xin[:, 0:512])
            nc.sync.dma_start(out=xim[:, 512:1024], in_=xin[:, 512:1024])
            nc.sync.dma_start(out=xim[:, 1024:2048], in_=xin[:, 1024:2048])

            # weight tiles streamed just-in-time; DMAs go on the second
            # HWDGE ring (scalar) so their
            # issue overlaps the xim DMA on the sync ring; chunk sizes grow
            # so the first matmuls are gated on as little data as possible
            wch = {}   # call l -> (tile, col index)
            c0 = 0
            prev = None
            for t, cs in enumerate(WCHUNKS):
                wt = cpool.tile([128, cs, 128], dt, tag=f"w{t}")
                if prev is not None:
                    # ladder: delay this chunk until the previous one is
                    # done (SDMA round-robin would otherwise steal early
                    # bandwidth from first-needed data); the probe write
                    # is overwritten by the DMA and only forces the dep
                    nc.vector.memset(wt[0:32, 0:1, 0:2].bitcast(mybir.dt.float32), 0.0)
                    nc.vector.tensor_copy(
                        wt[0:32, 0:1, 0:2].bitcast(mybir.dt.float32),
                        prev[0:32, 0:1, 0:2].bitcast(mybir.dt.float32),
                    )
                nc.scalar.dma_start(out=wt[:], in_=wd[:, c0 : c0 + cs, :])
                for j in range(cs):
                    wch[c0 + j] = (wt, j)
                c0 += cs
                prev = wt

            bias_sb = cpool.tile([COUT, 1], f32)
            nc.scalar.dma_start(out=bias_sb[:], in_=bd[:])

            # acc starts as broadcast(bias): the per-tile output is then
            # just a DMA of the finished acc slice (no ScalarE pass)
            acc = cpool.tile([COUT, L + 64], f32)
            nc.vector.memset(acc[:], 0.0)
            nc.scalar.activation(
                acc[:],
                acc[:],
                mybir.ActivationFunctionType.Identity,
                bias=bias_sb[:],
            )

            for T in range(NT):
                ps = pspool.tile([128, 512], f32)
                ncalls = 16 * (T + 1)
                for l in range(ncalls):
                    s = 512 * T - 32 * l
                    # columns below n0 only touch the zero padding -> trim
                    n0 = max(0, 32 * (l - 16 * T))
                    wt, j = wch[l]
                    nc.tensor.matmul(
                        ps[:, n0:512],
                        wt[:, j, :],
                        xim[:, s + n0 : s + 512],
                        start=(l == 0),
                        stop=(l == ncalls - 1),
                    )
                # fold tap-groups: psum[g*32+o, n] -> out[o, 512T + n + 4g];
                # each add spills up to 12 columns into the next tile's
                # region (or acc's pad for T=3), which that tile's DMA
                # picks up later -- exactly-once coverage per (col, g)
                for g in range(4):
                    nc.vector.tensor_add(
                        out=acc[:, 512 * T + 4 * g : 512 * T + 4 * g + 512],
                        in0=acc[:, 512 * T + 4 * g : 512 * T + 4 * g + 512],
                        in1=ps[32 * g : 32 * g + 32, :],
                    )
                nc.sync.dma_start(
                    out=od[:, 512 * T : 512 * T + 512],
                    in_=acc[:, 512 * T : 512 * T + 512],
                )

    nc.compile()
    return nc


def _gen_flipped_kernel(w1, b1, w2, b2, w3, b3):
    """SIREN generator with reversed positions -> Wf[o, i, j'] = k[o, i, L-1-j']."""
    pos = np.linspace(-1.0, 1.0, L, dtype=np.float32)[::-1].astype(np.float64)
    w1 = w1.astype(np.float64)
    w2 = w2.astype(np.float64)
    w3 = w3.astype(np.float64)
    h = np.sin(OMEGA * (w1[:, 0][:, None] * pos[None, :] + b1.astype(np.float64)[:, None]))
    h = np.sin(OMEGA * (w2 @ h + b2.astype(np.float64)[:, None]))
    k = w3 @ h + b3.astype(np.float64)[:, None]
    return k.reshape(COUT, CIN, L).astype(np.float32)


def _shuffle_weights(wf, npdt):
    """wf[o,i,j'] -> per tap-parity-half h: wt[p=dj*32+i, l, m=g*32+o]
    = wf[o, i, 32l + 16h + 4g + dj].

    The device pairs call l's weights with x-window column
    512T + n - 32l - dj and the unpack maps psum col n to
    out t = 512T + n + 4g.  With tap J = 32l + 16h + 4g + dj the correct
    x index is t - J = 512T + n - 32l - dj - 16h: the h=1 core therefore
    receives its input shifted right by 16 columns (see kernel()), which
    makes the device program identical on all cores.
    """
    outs = []
    for h in range(2):
        v = wf.reshape(COUT, CIN, NCH, 2, 4, 4)[:, :, :, h]   # [o,i,l,g,dj]
        v = v.transpose(4, 1, 2, 3, 0)                        # [dj,i,l,g,o]
        outs.append(np.ascontiguousarray(v.reshape(128, NCH, 128).astype(npdt)))
    return outs


def kernel(x, w1, b1, w2, b2, w3, b3, bias):
    global LAST_EXEC_NS, LAST_RESULTS
    x = np.ascontiguousarray(np.asarray(x, dtype=np.float32))
    bias = np.asarray(bias, dtype=np.float32)
    npdt = np.float16 if KDTYPE == "fp16" else np.float32

    wf = _gen_flipped_kernel(
        np.asarray(w1), np.asarray(b1), np.asarray(w2), np.asarray(b2),
        np.asarray(w3), np.asarray(b3),
    )  # [COUT, CIN, L]
    wds = _shuffle_weights(wf, npdt)

    if KDTYPE not in _NC:
        _NC[KDTYPE] = _build_nc(KDTYPE)

    # host-built im2col images: xim[dj*32+i, dj+t] = xc[i, t] where
    # xc = x for h=0 and x shifted right by 16 for h=1 (its taps are 16
    # later); columns beyond XIMW are never read and simply dropped
    xh = x.astype(npdt)
    xims = np.zeros((B, 2, 128, XIMW), dtype=npdt)
    for dj in range(4):
        blk = slice(32 * dj, 32 * dj + 32)
        xims[:, 0, blk, dj:XIMW] = xh[:, :, : L - dj]
        xims[:, 1, blk, dj + 16 : XIMW] = xh[:, :, : L - dj - 16]

    bias0 = np.ascontiguousarray(bias.reshape(COUT, 1))
    bias1 = np.zeros((COUT, 1), dtype=np.float32)

    in_maps = []
    for k in range(NCORES):
        b, h = k // 2, k % 2
        in_maps.append(
            {
                "xin": xims[b, h],
                "w": wds[h],
                "bias": bias0 if h == 0 else bias1,
            }
        )

    res = run_bass_kernel_spmd(
        _NC[KDTYPE], in_maps, core_ids=list(range(NCORES)), trace=TRACE
    )
    LAST_RESULTS = res
    LAST_EXEC_NS = res.exec_time_ns

    out = np.empty((B, COUT, L), dtype=np.float32)
    for b in range(B):
        out[b] = res.results[2 * b]["out"] + res.results[2 * b + 1]["out"]
    return out

